# revision 23
# baseline (speedup 1.0000x reference)
"""Trainium2 Bass kernel for BERT word-pooling (segment mean + CLS).

Computation (matches the jax reference):
  hidden = mean over 4 layers of hidden_layers[4, B, T, D]
  per example b: word_emb[j] = mean of hidden[b, t] over tokens with
  word_ids[b, t] == j (j < 100; 100 is the pad sentinel), empty words -> 0
  output rows per example: [cls = hidden[b, 0], word_emb[0..99]]
  -> [B*101, D]

Strategy: pure data parallel, 4 examples per core across 8 cores. The
problem is HBM-bandwidth bound, so the host ships layers 0+1 as f16 and
layers 2+3 as per-token-scaled int8 (q = rint(x / s_t), s_t =
max|x_{2,3}| over the token / 127): 13.6 MB/core of DRAM reads vs 33.6
f32. The dtype mix balances the two on-chip walls: an all-int8 shipment
would bottleneck on the DVE (int8 adds run at ~118 G elem/s), while
all-f16 bottlenecks on HBM. Quantization noise lands ~5e-3 rel, well
inside the 2e-2 budget, and the reconstruction is exact-linear: the
per-token scale is folded into a second one-hot matrix (S2 = S * s_t,
built on the ACT engine), and integer pair sums q2 + q3 <= 254 are
exact in f16.

The int8 pairs ride the scalar HWDGE ring (4.2 MB - finishes early, so
the slow int8 adds get slack); the f16 pairs ride the sync ring as
1 MiB half-example DMAs so the dependent chain after the last-landing
MiB is short. All DMAs are fully contiguous 4-8 KiB/partition lines.

Per example, on device:
  q23 = q2 + q3 (DVE, int8 -> f16)    h01 = l0 + l1 (DVE, f16, halves)
  psum[j, d] = sum_t S2[t, j] * q23[t, d] + sum_t S[t, j] * h01[t, d]
  counts[j]  = sum_t S[t, j] * 4.0
  out[j, d]  = psum[j, d] / max(counts[j], 4)  (= segment mean over the
               4-layer sum; d-half 0 scaled on DVE, half 1 on ACT)
The one-hot columns are shifted by +1 (word j -> column j+1) and column 0
marks token 0, so the CLS row falls out of the same matmul + scale
pipeline (its count is 1 -> scale 1/4) and rows 0..100 of the result tile
are exactly one example's output block, stored with a single DMA.
"""

import sys

for _p in ("/opt/trn_rl_repo", "/opt/trn_rl_repo/concourse"):
    if _p not in sys.path:
        sys.path.append(_p)

from contextlib import ExitStack

import numpy as np

import concourse.bacc as bacc
import concourse.bass as bass
import concourse.tile as tile
from concourse import mybir
from concourse.bass_utils import run_bass_kernel_spmd

B, T, D, W = 32, 512, 1024, 100
N_CORES = 8
BL = B // N_CORES          # examples per core
NT = T // 128              # token chunks; token t = p*NT + c (p-major)
NH = NT // 2               # token chunks per f16 half-DMA
ND = D // 512              # 512-wide d chunks (one PSUM bank each)
OUT_PAD = 128              # padded per-example output rows (contiguous stores)
OUT_ROWS = BL * OUT_PAD    # output rows per core (kernel-side, padded)
PREFETCH = 3               # examples of load tiles in flight

_f32 = mybir.dt.float32
_f16 = mybir.dt.float16
_i32 = mybir.dt.int32
_i8 = mybir.dt.int8


def _build_program() -> bass.Bass:
    # Bacc (not raw Bass): its compile() runs generate_event_semaphores,
    # which splits multi-wait DMAs (DMA instrs have a single HW wait slot).
    nc = bacc.Bacc(
        "TRN2", target_bir_lowering=False, debug=False, num_devices=N_CORES
    )
    hf = nc.declare_dram_parameter("hf", [2, BL, T, D], _f16, isOutput=False)
    hq = nc.declare_dram_parameter("hq", [2, BL, T, D], _i8, isOutput=False)
    # word ids (as exact f32 - is_equal wants f32 anyway) and scales,
    # pre-transposed on the host to partition-major and packed into ONE
    # [128, 32] tensor: a single 128B/partition DMA instead of per-example
    # 16B/partition descriptor showers that clog a ring
    meta = nc.declare_dram_parameter(
        "meta", [128, 2 * BL * NT], _f32, isOutput=False
    )
    out = nc.declare_dram_parameter("out", [OUT_ROWS, D], _f16, isOutput=True)

    with tile.TileContext(nc) as tc, ExitStack() as ctx:
        const = ctx.enter_context(tc.tile_pool(name="const", bufs=1))
        hpool = ctx.enter_context(tc.tile_pool(name="hpool", bufs=PREFETCH))
        sumpool = ctx.enter_context(tc.tile_pool(name="sumpool", bufs=2))
        spool = ctx.enter_context(tc.tile_pool(name="spool", bufs=2))
        vpool = ctx.enter_context(tc.tile_pool(name="vpool", bufs=2))
        opool = ctx.enter_context(tc.tile_pool(name="opool", bufs=2))
        psum = ctx.enter_context(tc.tile_pool(name="psum", bufs=2, space="PSUM"))

        # column j holds value j-1 in every partition (f32: is_equal wants f32
        # operands). Word j then lands in one-hot column j+1, and column 0
        # (value -1, never a word id) is reserved for the CLS marker, so the
        # out_sb rows 0..100 are exactly one example's output block.
        iota_i = const.tile([128, 128], _i32)
        nc.gpsimd.iota(iota_i[:], [[1, 128]], base=-1, channel_multiplier=0)
        iota_t = const.tile([128, 128], _f32)
        nc.vector.tensor_copy(iota_t[:], iota_i[:])
        # counts rhs: 4.0 so counts come out as 4*count (the layer factor)
        ones4 = const.tile([128, 1], _f16)
        nc.vector.memset(ones4[:], 4.0)

        # all examples' word ids + scales in one small DMA at the head of
        # the scalar ring (cols 0..BL*NT-1: word ids, cols BL*NT..: scales)
        metm = const.tile([128, 2 * BL * NT], _f32)
        nc.scalar.dma_start(metm[:], meta[:, :])

        # per-example state, filled in by the emission units below
        st = [dict() for _ in range(BL)]

        def issue_loads(b):
            # int8 pair (layers 2,3): one 1 MiB DMA on the scalar ring.
            # Partition p holds tokens p*NT..p*NT+NT-1 back to back. All 4
            # int8 tiles get their own buffer so every int8 load flies early.
            tq = hpool.tile([128, 2, NT * D], _i8, tag="q23", name="q23", bufs=BL)
            nc.scalar.dma_start(
                tq[:],
                hq[:, b].rearrange("l (p c) m -> p l (c m)", p=128),
            )
            # f16 pair (layers 0,1): two 1 MiB half-DMAs on the sync ring
            # (token chunks 0-1, then 2-3), so the last-landing transfer
            # gates only half an example's adds/matmuls. The very last half
            # (example BL-1, tokens 2-3) is further split into two 512KB
            # quarter-DMAs: the dependent chain behind the final transfer
            # is then only a quarter-add + 2 matmuls.
            ths = []
            src = hf[:, b].rearrange("l (p c) m -> p l c m", p=128)
            for h in range(2):
                th = hpool.tile(
                    [128, 2, NH, D], _f16, tag=f"f01h{h}", name=f"f01h{h}",
                    bufs=BL,
                )
                if b == BL - 1 and h == 1:
                    for qtr in range(2):
                        nc.sync.dma_start(
                            th[:, :, qtr, :],
                            src[:, :, h * NH + qtr, :],
                        )
                else:
                    nc.sync.dma_start(th[:], src[:, :, h * NH : (h + 1) * NH, :])
                ths.append(th)
            st[b].update(tq=tq, ths=ths)

        def unit_s(b):
            # one-hot S per token chunk: S[t, j] = (wid[t] == j-1) in f16
            # (counts + f16-pair matmuls), and S2[t, j] = S[t, j] * s_t on
            # the ACT engine (int8-pair matmuls: dequant folded into the
            # one-hot, never into the bulk data). Plus the counts matmul.
            s_tiles, s2_tiles = [], []
            for c in range(NT):
                s_c = spool.tile([128, 128], _f16, tag=f"s{c}", name=f"s{c}")
                nc.vector.tensor_scalar(
                    s_c[:], iota_t[:],
                    metm[:, b * NT + c : b * NT + c + 1], None,
                    mybir.AluOpType.is_equal,
                )
                if c == 0:
                    # CLS marker: token 0 (p=0, c=0) also feeds output row 0
                    nc.vector.memset(s_c[0:1, 0:1], 1.0)
                s2_c = spool.tile([128, 128], _f16, tag=f"s2{c}", name=f"s2{c}")
                nc.scalar.activation(
                    s2_c[:], s_c[:], mybir.ActivationFunctionType.Copy,
                    scale=metm[:, (BL + b) * NT + c : (BL + b) * NT + c + 1],
                )
                s_tiles.append(s_c)
                s2_tiles.append(s2_c)
            counts_ps = psum.tile([128, 1], _f32, tag="counts")
            for c in range(NT):
                nc.tensor.matmul(
                    counts_ps[:], s_tiles[c][:], ones4[:],
                    start=(c == 0), stop=(c == NT - 1),
                )
            pss = [
                psum.tile([128, 512], _f32, tag=f"ps{d}", name=f"ps{d}")
                for d in range(ND)
            ]
            st[b].update(s=s_tiles, s2=s2_tiles, counts=counts_ps, pss=pss)

        def unit_i8(b):
            # int8 pair sum (exact: |q2 + q3| <= 254) + its matmuls
            tq, s2_tiles, pss = st[b]["tq"], st[b]["s2"], st[b]["pss"]
            q23 = sumpool.tile([128, NT * D], _f16, tag="q23s", name="q23s")
            nc.vector.tensor_tensor(
                q23[:], tq[:, 0, :], tq[:, 1, :], mybir.AluOpType.add
            )
            for d in range(ND):
                for c in range(NT):
                    nc.tensor.matmul(
                        pss[d][:], s2_tiles[c][:],
                        q23[:, c * D + d * 512 : c * D + d * 512 + 512],
                        start=(c == 0), stop=False,
                    )

        def unit_f16(b, h):
            # f16 half-pair sum + its matmuls; the very last half is
            # processed per 512KB quarter to match its quarter-DMAs
            th, s_tiles, pss = st[b]["ths"][h], st[b]["s"], st[b]["pss"]
            h01 = sumpool.tile([128, NH * D], _f16, tag=f"h01{h}", name=f"h01{h}")
            split = b == BL - 1 and h == 1
            for part in range(2) if split else (None,):
                if part is None:
                    nc.vector.tensor_tensor(
                        h01[:], th[:, 0, :, :], th[:, 1, :, :],
                        mybir.AluOpType.add,
                    )
                    cis = range(NH)
                else:
                    nc.vector.tensor_tensor(
                        h01[:, part * D : (part + 1) * D],
                        th[:, 0, part, :], th[:, 1, part, :],
                        mybir.AluOpType.add,
                    )
                    cis = (part,)
                for d in range(ND):
                    for ci in cis:
                        c = h * NH + ci
                        nc.tensor.matmul(
                            pss[d][:], s_tiles[c][:],
                            h01[:, ci * D + d * 512 : ci * D + d * 512 + 512],
                            start=False,
                            stop=(h == 1 and ci == NH - 1),
                        )

        def unit_fin(b):
            # 1/max(4*count, 4) then the PSUM scale (on ACT; for the last
            # example d-half 0 goes to the otherwise-drained DVE so the two
            # halves run concurrently) and the store. One fully-contiguous
            # 256KB store per example (output padded to 128 rows; the host
            # slices rows 0..100); it rides the scalar ring, whose small
            # int8 queue drained early.
            counts_ps, pss = st[b]["counts"], st[b]["pss"]
            scale_t = vpool.tile([128, 1], _f32, tag="scale")
            recip_t = vpool.tile([128, 1], _f32, tag="recip")
            nc.vector.tensor_scalar_max(scale_t[:], counts_ps[:], 4.0)
            nc.vector.reciprocal(recip_t[:], scale_t[:])
            out_sb = opool.tile([128, D], _f16, tag="out_sb", name="out_sb")
            rows = slice(b * OUT_PAD, (b + 1) * OUT_PAD)
            if b == BL - 1:
                # last example: d-half 0 on the (by now drained) DVE in
                # parallel with d-half 1 on ACT, each stored as soon as it
                # is scaled
                nc.vector.tensor_scalar(
                    out_sb[:, 0:512], pss[0][:], recip_t[:, 0:1], None,
                    mybir.AluOpType.mult,
                )
                nc.scalar.dma_start(out[rows, 0:512], out_sb[:, 0:512])
                nc.scalar.activation(
                    out_sb[:, 512:1024], pss[1][:],
                    mybir.ActivationFunctionType.Copy, scale=recip_t[:, 0:1],
                )
                nc.scalar.dma_start(out[rows, 512:1024], out_sb[:, 512:1024])
            else:
                for d in range(ND):
                    nc.scalar.activation(
                        out_sb[:, d * 512 : (d + 1) * 512], pss[d][:],
                        mybir.ActivationFunctionType.Copy, scale=recip_t[:, 0:1],
                    )
                nc.scalar.dma_start(out[rows, :], out_sb[:])

        # Emission order matched to DMA arrival order: the int8 ring (4.2MB)
        # drains by ~1/3 of the stream while f16 halves trickle in until the
        # end, so int8 adds of later examples are interleaved between the
        # f16 halves of earlier ones. This keeps the DVE (the busiest
        # engine) fed in arrival order and the dependent tail short.
        for b in range(BL):
            issue_loads(b)
        for step in (
            ("s", 0), ("i8", 0), ("f16", 0, 0),
            ("s", 1), ("f16", 0, 1), ("i8", 1), ("f16", 1, 0), ("fin", 0),
            ("s", 2), ("i8", 2), ("f16", 1, 1), ("fin", 1),
            ("s", 3), ("i8", 3), ("f16", 2, 0), ("f16", 2, 1), ("fin", 2),
            ("f16", 3, 0), ("f16", 3, 1), ("fin", 3),
        ):
            kind = step[0]
            if kind == "s":
                unit_s(step[1])
            elif kind == "i8":
                unit_i8(step[1])
            elif kind == "f16":
                unit_f16(step[1], step[2])
            else:
                unit_fin(step[1])

    nc.compile()
    return nc


_PROGRAM = None
LAST_RESULTS = None   # BassKernelResults of the most recent run (for test.py)
TRACE = False         # set True from test.py to capture an NTFF profile


def _get_program() -> bass.Bass:
    global _PROGRAM
    if _PROGRAM is None:
        _PROGRAM = _build_program()
    return _PROGRAM


def kernel(hidden_layers, word_ids, num_words=W, **_ignored) -> np.ndarray:
    global LAST_RESULTS
    h = np.asarray(hidden_layers, dtype=np.float32)
    word_ids = np.asarray(word_ids, dtype=np.int32)
    assert h.shape == (4, B, T, D), h.shape
    assert word_ids.shape == (B, T), word_ids.shape
    assert int(num_words) == W, num_words

    # layers 0,1 -> f16; layers 2,3 -> per-token symmetric int8 (scale
    # shared across the two layers so integer pair sums stay exact)
    hf = h[0:2].astype(np.float16)
    s = np.max(np.abs(h[2:4]), axis=(0, 3)) / 127.0     # [B, T]
    s = np.maximum(s, 1e-8).astype(np.float32)
    q = np.rint(h[2:4] * (1.0 / s)[None, :, :, None]).astype(np.int8)

    def meta_t(x):
        # [BL, T] -> partition-major [128, BL*NT]: out[p, b*NT+c] = x[b, p*NT+c]
        return x.reshape(BL, 128, NT).transpose(1, 0, 2).reshape(128, BL * NT)

    in_maps = []
    for i in range(N_CORES):
        sl = slice(i * BL, (i + 1) * BL)
        in_maps.append(
            {
                "hf": np.ascontiguousarray(hf[:, sl]),
                "hq": np.ascontiguousarray(q[:, sl]),
                "meta": np.ascontiguousarray(
                    np.concatenate(
                        [
                            meta_t(word_ids[sl].astype(np.float32)),
                            meta_t(s[sl]),
                        ],
                        axis=1,
                    )
                ),
            }
        )

    res = run_bass_kernel_spmd(
        _get_program(), in_maps, core_ids=list(range(N_CORES)), trace=TRACE
    )
    LAST_RESULTS = res
    # kernel output is padded to 128 rows per example; keep rows 0..100
    outs = [
        res.results[i]["out"]
        .reshape(BL, OUT_PAD, D)[:, : W + 1, :]
        .reshape(-1, D)
        .astype(np.float32)
        for i in range(N_CORES)
    ]
    return np.concatenate(outs, axis=0)
